# revision 1
# baseline (speedup 1.0000x reference)
"""CTC loss (keras ctc_batch_cost semantics) on 8 Trainium2 NeuronCores.

Data-parallel over batch: 1024 samples -> 8 cores x 128 samples
(one sample per SBUF partition).  Host prep is integer-only (gather
index tables + skip masks); all float work runs on device.

Device pipeline (per core, per 128-step T-half):
  A. load y_pred [tau,c] sample tiles, ACT-cast fp32->bf16 with +EPS,
     DMA-xbar transpose to [c,tau], store rows to an HBM scratch.
  B. dma_gather: rows (b, c=label_j) and (b, blank) -> ptil[b, (blk, tau)]
     in one shot (per-sample label gathers via int16 row indices).
  C. per-column scale: g = max over a 9-block subset, ginv = e^rhat/g
     (fp32 reciprocal, bf16); ptil *= ginv (GPSIMD).
  D. s-sweep over the 129-row extended CTC lattice: each row's
     recursion v_t = (e_t + v_{t-1}) * p_t is ONE tensor_tensor_scan
     along the free dim; row coupling e_t = v^{s-1}_{t-1} + m*v^{s-2}_{t-1}
     is one scalar_tensor_tensor (odd rows) or a shifted view (even).
     Probability domain, per-half max-renorm keeps fp32 range.
Host assembles loss = -(log lsum + sum log bmax - sum log ginv) in f64.
"""
from contextlib import ExitStack

import numpy as np
import ml_dtypes

import concourse.bass as bass
import concourse.tile as tile
from concourse import bacc, mybir
from concourse.bass_utils import run_bass_kernel_spmd

F32 = mybir.dt.float32
BF16 = mybir.dt.bfloat16
I16 = mybir.dt.int16
AF = mybir.ActivationFunctionType
ALU = mybir.AluOpType

B, T, C, L = 1024, 256, 128, 64
S = 2 * L + 1          # 129 extended states
NBLK = L + 1           # 64 label blocks + 1 blank block
BLANK = C - 1
EPS = 1e-7
RHAT = 0.4             # per-step prob boost exp(RHAT) centers chunk decay
TC = 128               # scan chunk length == tau-half
NCH = T // TC          # 2
W = T + 1              # Treg slot width: col0 = v_{-1}, col 1+t = v_t
SLOTS = S + 2          # 2 permanent zero rows + 129 state rows
PB = 128               # samples per core
NCORES = 8
SGRP = 4               # samples per load/cast group


def _host_prep(y_true_shard: np.ndarray):
    yt = y_true_shard.astype(np.int64)
    idx_flat = np.empty(NBLK * PB, np.int32)
    barange = np.arange(PB) * C
    for j in range(L):
        idx_flat[j * PB:(j + 1) * PB] = barange + yt[:, j]
    idx_flat[L * PB:] = barange + BLANK
    table16 = idx_flat.reshape(NBLK * PB // 16, 16).T      # [16, 520]
    idxs = np.tile(table16, (8, 1)).astype(np.int16)        # [128, 520]
    m01 = np.ones((PB, L), np.float32)
    m01[:, 1:] = (yt[:, 1:] != yt[:, :-1]).astype(np.float32)
    m01[:, 0] = 0.0
    return {"idxs": idxs, "m01": m01}


def _emit(ctx: ExitStack, tc: tile.TileContext, y_in, idxs_in, m01_in,
          raw_out, ginv_out):
    nc = tc.nc

    persist = ctx.enter_context(tc.tile_pool(name="persist", bufs=1))
    stage = ctx.enter_context(tc.tile_pool(name="stage", bufs=4))
    trp = ctx.enter_context(tc.tile_pool(name="trp", bufs=8))
    cpool = ctx.enter_context(tc.tile_pool(name="cbuf", bufs=4))
    scratch = ctx.enter_context(tc.tile_pool(name="scratch", bufs=2))
    dram = ctx.enter_context(tc.tile_pool(name="dram", bufs=1, space="DRAM"))

    idxs = persist.tile([PB, NBLK * PB // 16], I16)
    nc.sync.dma_start(idxs[:], idxs_in[:])
    m01 = persist.tile([PB, L], F32)
    nc.sync.dma_start(m01[:], m01_in[:])

    treg_t = persist.tile([PB, SLOTS * W], F32)
    nc.gpsimd.memset(treg_t[:], 0.0)
    raw = persist.tile([PB, NCH], F32)
    epsb = persist.tile([PB, 1], F32)
    nc.vector.memset(epsb[:], EPS)

    ytT, ptil, ginvb = [], [], []
    for h in range(NCH):
        ytT_h = dram.tile([PB * C, TC], BF16, tag=f"ytT{h}")
        ptil_h = persist.tile([PB, NBLK * TC], BF16, tag=f"ptil{h}")
        ginvb_h = persist.tile([PB, TC], BF16, tag=f"ginvb{h}")
        ytT.append(ytT_h); ptil.append(ptil_h); ginvb.append(ginvb_h)

    def phase_abc(h):
        # A: load + cast(+eps) + xbar transpose + store rows to HBM
        for g in range(PB // SGRP):
            b0 = g * SGRP
            ld = stage.tile([PB, SGRP * C], F32, tag="ld")
            nc.sync.dma_start(
                ld[:].rearrange("p (b c) -> p b c", b=SGRP),
                y_in[b0:b0 + SGRP, h * TC:(h + 1) * TC, :]
                .rearrange("b t c -> t b c"))
            bf = stage.tile([PB, SGRP * C], BF16, tag="bf")
            nc.scalar.activation(bf[:], ld[:], AF.Identity, bias=epsb[:, 0:1])
            for i in range(SGRP):
                b = b0 + i
                tr = trp.tile([C, TC], BF16, tag="tr")
                eng = nc.sync if (i % 2 == 0) else nc.scalar
                eng.dma_start_transpose(tr[:], bf[:, i * C:(i + 1) * C])
                eng2 = nc.scalar if (i % 2 == 0) else nc.sync
                eng2.dma_start(ytT[h][b * C:(b + 1) * C, :], tr[:])
        # B: gathers (8 label-block groups + blank) across SWDGE queues
        for q in range(8):
            nc.gpsimd.dma_gather(
                ptil[h][:, q * 8 * TC:(q + 1) * 8 * TC]
                .rearrange("p (i e) -> p i e", e=TC),
                ytT[h][:],
                idxs[:, 64 * q:64 * q + 64],
                num_idxs=8 * PB, num_idxs_reg=8 * PB,
                elem_size=TC, queue_num=q % 4)
        nc.gpsimd.dma_gather(
            ptil[h][:, L * TC:NBLK * TC]
            .rearrange("p (i e) -> p i e", e=TC),
            ytT[h][:],
            idxs[:, 512:520],
            num_idxs=PB, num_idxs_reg=PB,
            elem_size=TC, queue_num=0)
        # C: per-column scale
        p3 = ptil[h][:].rearrange("p (blk t) -> p t blk", blk=NBLK)
        gmax = scratch.tile([PB, TC], F32, tag="gmax")
        nc.vector.tensor_reduce(gmax[:], p3[:, :, 0:NBLK:8],
                                axis=mybir.AxisListType.X, op=ALU.max)
        nc.vector.tensor_scalar_mul(gmax[:], gmax[:], float(np.exp(-RHAT)))
        ginv32 = scratch.tile([PB, TC], F32, tag="ginv32")
        nc.vector.reciprocal(ginv32[:], gmax[:])
        nc.vector.tensor_copy(ginvb[h][:], ginv32[:])
        nc.sync.dma_start(ginv_out[:, h * TC:(h + 1) * TC], ginvb[h][:])
        for blk in range(NBLK):
            sl = ptil[h][:, blk * TC:(blk + 1) * TC]
            nc.gpsimd.tensor_mul(sl, sl, ginvb[h][:])

    def sb(s):  # Treg slot base col
        return (s + 2) * W

    def phase_d(k):
        t0 = k * TC
        if k > 0:
            start = 2 * W + t0
            bcols = treg_t[:, start:start + (S - 1) * W + 1:W]
            nc.vector.tensor_reduce(raw[:, k:k + 1], bcols,
                                    axis=mybir.AxisListType.X, op=ALU.max)
            rinv = scratch.tile([PB, 1], F32, tag="rinv")
            nc.vector.reciprocal(rinv[:], raw[:, k:k + 1])
            nc.vector.tensor_scalar_mul(bcols, bcols, rinv[:])
        for s in range(S):
            base = sb(s)
            if s % 2 == 1:
                j = (s - 1) // 2
                c = cpool.tile([PB, TC], F32, tag="c")
                nc.vector.scalar_tensor_tensor(
                    c[:],
                    treg_t[:, sb(s - 2) + t0: sb(s - 2) + t0 + TC],
                    m01[:, j:j + 1],
                    treg_t[:, sb(s - 1) + t0: sb(s - 1) + t0 + TC],
                    op0=ALU.mult, op1=ALU.add,
                )
                d0 = c[:]
                blk = j
            else:
                d0 = treg_t[:, sb(s - 1) + t0: sb(s - 1) + t0 + TC]
                blk = L
            # chunk 0: immediate initial (col-0 cells stay 0 — they feed the
            # t=0 coupling reads of rows s+1, s+2)
            if k == 0:
                initial = 1.0 if s <= 1 else 0.0
            else:
                initial = treg_t[:, base + t0: base + t0 + 1]
            nc.vector.tensor_tensor_scan(
                treg_t[:, base + 1 + t0: base + 1 + t0 + TC],
                d0,
                ptil[k][:, blk * TC:(blk + 1) * TC],
                initial,
                op0=ALU.add, op1=ALU.mult,
            )

    for h in range(NCH):
        phase_abc(h)
    for k in range(NCH):
        phase_d(k)

    b127 = sb(127) + T
    b128 = sb(128) + T
    nc.vector.tensor_add(raw[:, 0:1], treg_t[:, b127:b127 + 1],
                         treg_t[:, b128:b128 + 1])
    nc.sync.dma_start(raw_out[:], raw[:])


_CACHE: dict = {}


def _build():
    nc = bacc.Bacc("TRN2", target_bir_lowering=False, debug=False,
                   num_devices=NCORES, num_swdge_queues=4)
    y_in = nc.dram_tensor("ypred", [PB, T, C], F32, kind="ExternalInput").ap()
    idxs_in = nc.dram_tensor("idxs", [PB, NBLK * PB // 16], I16,
                             kind="ExternalInput").ap()
    m01_in = nc.dram_tensor("m01", [PB, L], F32, kind="ExternalInput").ap()
    raw_out = nc.dram_tensor("raw", [PB, NCH], F32, kind="ExternalOutput").ap()
    ginv_out = nc.dram_tensor("ginv", [PB, T], BF16, kind="ExternalOutput").ap()
    with tile.TileContext(nc) as tcx:
        with ExitStack() as ctx:
            _emit(ctx, tcx, y_in, idxs_in, m01_in, raw_out, ginv_out)
    nc.compile()
    return nc


def _run(in_maps, **kwargs):
    if "nc" not in _CACHE:
        _CACHE["nc"] = _build()
    return run_bass_kernel_spmd(_CACHE["nc"], in_maps,
                                core_ids=list(range(NCORES)), **kwargs)


def kernel(y_true: np.ndarray, y_pred: np.ndarray, **run_kwargs) -> np.ndarray:
    assert y_pred.shape == (B, T, C), y_pred.shape
    in_maps = []
    for c in range(NCORES):
        sl = slice(c * PB, (c + 1) * PB)
        prep = _host_prep(y_true[sl])
        in_maps.append({"ypred": np.ascontiguousarray(y_pred[sl], np.float32),
                        "idxs": prep["idxs"], "m01": prep["m01"]})
    res = _run(in_maps, **run_kwargs)
    raw = np.concatenate([res.results[c]["raw"] for c in range(NCORES)], axis=0)
    ginv = np.concatenate([res.results[c]["ginv"] for c in range(NCORES)],
                          axis=0).astype(np.float64)
    lng = np.log(ginv).sum(axis=1)
    val = np.log(raw[:, 0].astype(np.float64))
    val += np.log(raw[:, 1:].astype(np.float64)).sum(axis=1)
    loss = -(val - lng)
    if run_kwargs:
        kernel.last_results = res  # expose trace info to test harness
    return loss[:, None].astype(np.float32)



# revision 3
# speedup vs baseline: 1.1599x; 1.1599x over previous
"""CTC loss (keras ctc_batch_cost semantics) on 8 Trainium2 NeuronCores.

Data-parallel over batch: 1024 samples -> 8 cores x 128 samples.
Host prep is integer-only (gather index tables + skip masks); all float
work runs on device.

Device pipeline (per core), built to minimize DMA instruction count
(HWDGE issue is ~630ns serialized per dma_start) and DRAM round-trips:

  A. 4 big loads (one per (half, 64-sample group)) of y_pred in t-major
     layout [t, (b, c)] - each descriptor is a contiguous 512B class row,
     so the loads run at full HBM bandwidth.
  B. ap_gather on GPSIMD: per t-partition, gather the 65 needed class
     columns (64 labels + blank) per sample entirely on-chip -> gout
     [t, (blk, b)].  Indices depend only on (sample, label), never on t,
     so the per-16-partition shared index list is valid.
  C. per-column scale in t-major: gmax over a 9-block subset, one
     broadcast scalar_tensor_tensor applies (p + EPS) * (e^RHAT / gmax).
  D. 65 PE transposes per (half, group): [t, b] -> PSUM [b, t]; Scalar
     engine copies PSUM -> ptil [b, blk, T] with bf16 downcast.
  E. s-sweep over the 129-row extended CTC lattice: each row's recursion
     v_t = (e_t + v_{t-1}) * p_t is ONE tensor_tensor_scan over the full
     T=256 (no mid-chunk renorm; per-column scaling keeps fp32 range,
     validated max |v| ~ 6.7e20, min lsum ~ 1e-27).  Row storage is a
     5-slot ring buffer instead of the full [129, T] series.
Host assembles loss = -(log lsum - sum log ginv) in f64.
"""
from contextlib import ExitStack

import numpy as np

import concourse.bass as bass
import concourse.tile as tile
from concourse import bacc, mybir
from concourse.bass_utils import run_bass_kernel_spmd
from concourse.masks import make_identity

F32 = mybir.dt.float32
BF16 = mybir.dt.bfloat16
I16 = mybir.dt.int16
AF = mybir.ActivationFunctionType
ALU = mybir.AluOpType

B, T, C, L = 1024, 256, 128, 64
S = 2 * L + 1          # 129 extended states
NBLK = L + 1           # 64 label blocks + 1 blank block
BLANK = C - 1
EPS = 1e-7
RHAT = 0.2             # per-step prob boost: ginv = e^RHAT / gmax
TC = 128               # t-steps per load half
NCH = T // TC          # 2
PB = 128               # samples per core
G = 64                 # samples per load/gather group
NG = PB // G           # 2
NIDX = NBLK * G        # gather indices per group (4160)
RING = 5               # ring slots: 1 zero slot + 4 live state series
W = T + 1              # ring slot width: col0 = v_{-1} = 0, col 1+t = v_t
NCORES = 8


def _host_prep(y_true_shard: np.ndarray):
    yt = y_true_shard.astype(np.int64)
    tables = []
    for g in range(NG):
        cls = np.full((NBLK, G), BLANK, np.int64)        # [blk, i]
        cls[:L, :] = yt[g * G:(g + 1) * G, :].T
        idx_flat = (np.arange(G) * C)[None, :] + cls     # i*C + class
        t16 = idx_flat.reshape(NIDX // 16, 16).T         # [16, 260]
        tables.append(np.tile(t16, (8, 1)))              # [128, 260]
    idxs = np.concatenate(tables, axis=1).astype(np.int16)  # [128, 520]
    m01 = np.ones((PB, L), np.float32)
    m01[:, 1:] = (yt[:, 1:] != yt[:, :-1]).astype(np.float32)
    m01[:, 0] = 0.0
    return {"idxs": idxs, "m01": m01}


def _emit(ctx: ExitStack, tc: tile.TileContext, y_in, idxs_in, m01_in,
          raw_out, ginv_out):
    nc = tc.nc

    persist = ctx.enter_context(tc.tile_pool(name="persist", bufs=1))
    srcp = ctx.enter_context(tc.tile_pool(name="src", bufs=2))
    goutp = ctx.enter_context(tc.tile_pool(name="gout", bufs=2))
    small = ctx.enter_context(tc.tile_pool(name="small", bufs=4))
    cpool = ctx.enter_context(tc.tile_pool(name="cbuf", bufs=4))
    psum = ctx.enter_context(
        tc.tile_pool(name="ps", bufs=8, space=bass.MemorySpace.PSUM))

    idxs = persist.tile([PB, NG * (NIDX // 16)], I16)
    nc.sync.dma_start(idxs[:], idxs_in[:])
    m01 = persist.tile([PB, L], F32)
    nc.sync.dma_start(m01[:], m01_in[:])

    ident = persist.tile([128, 128], F32)
    make_identity(nc, ident[:])

    # ptil[b, blk, t_global] bf16; ring of fp32 state series
    ptil = persist.tile([PB, NBLK * T], BF16)
    ring = persist.tile([PB, RING * W], F32)
    nc.gpsimd.memset(ring[:], 0.0)
    ginv_all = persist.tile([128, NCH * PB], F32)   # [t, (h, g, b)]
    raw = persist.tile([PB, 1], F32)

    # ---- feed: load -> gather -> scale -> transpose -> copy ----
    for h in range(NCH):
        for g in range(NG):
            src = srcp.tile([128, G * C], F32, tag="src")
            nc.sync.dma_start(
                src[:].rearrange("p (b c) -> p b c", b=G),
                y_in[g * G:(g + 1) * G, h * TC:(h + 1) * TC, :]
                .rearrange("b t c -> t b c"))
            gout = goutp.tile([128, NIDX], F32, tag="gout")
            nc.gpsimd.ap_gather(
                gout[:], src[:], idxs[:, g * (NIDX // 16):(g + 1) * (NIDX // 16)],
                channels=128, num_elems=G * C, d=1, num_idxs=NIDX)
            # column scale: gmax over blk subset {0,8,...,64}
            g3 = gout[:].rearrange("p (blk b) -> p b blk", blk=NBLK)
            gmax = small.tile([128, G], F32, tag="gmax")
            nc.vector.tensor_reduce(gmax[:], g3[:, :, 0:NBLK:8],
                                    axis=mybir.AxisListType.X, op=ALU.max)
            nc.vector.tensor_scalar_mul(gmax[:], gmax[:], float(np.exp(-RHAT)))
            gslice = ginv_all[:, (h * PB + g * G):(h * PB + (g + 1) * G)]
            nc.vector.reciprocal(gslice, gmax[:])
            nc.vector.scalar_tensor_tensor(
                gout[:].rearrange("p (blk b) -> p blk b", blk=NBLK),
                gout[:].rearrange("p (blk b) -> p blk b", blk=NBLK),
                EPS,
                gslice[:, None, :].broadcast_to([128, NBLK, G]),
                op0=ALU.add, op1=ALU.mult)
            for blk in range(NBLK):
                ps = psum.tile([G, TC], F32, tag="ps")
                nc.tensor.transpose(
                    ps[:], gout[:, blk * G:(blk + 1) * G], ident[:])
                nc.scalar.activation(
                    ptil[g * G:(g + 1) * G, blk * T + h * TC: blk * T + (h + 1) * TC],
                    ps[:], AF.Identity)

    nc.sync.dma_start(ginv_out[:], ginv_all[:])

    # ---- phase D: 129 states, one full-T scan each, 5-slot ring ----
    def slot(s):  # ring slot columns for state s (s = -1 is the zero slot)
        r = (s + 1) % RING
        return ring[:, r * W:(r + 1) * W]

    for s in range(S):
        if s % 2 == 1:
            j = (s - 1) // 2
            c = cpool.tile([PB, T], F32, tag="c")
            nc.vector.scalar_tensor_tensor(
                c[:], slot(s - 2)[:, 0:T], m01[:, j:j + 1], slot(s - 1)[:, 0:T],
                op0=ALU.mult, op1=ALU.add)
            d0 = c[:]
            blk = j
        else:
            d0 = slot(s - 1)[:, 0:T]
            blk = L
        nc.vector.tensor_tensor_scan(
            slot(s)[:, 1:1 + T], d0, ptil[:, blk * T:(blk + 1) * T],
            1.0 if s <= 1 else 0.0,
            op0=ALU.add, op1=ALU.mult)

    nc.vector.tensor_add(raw[:, 0:1], slot(S - 2)[:, T:T + 1],
                         slot(S - 1)[:, T:T + 1])
    nc.sync.dma_start(raw_out[:], raw[:])


_CACHE: dict = {}


def _build():
    nc = bacc.Bacc("TRN2", target_bir_lowering=False, debug=False,
                   num_devices=NCORES, num_swdge_queues=4)
    y_in = nc.dram_tensor("ypred", [PB, T, C], F32, kind="ExternalInput").ap()
    idxs_in = nc.dram_tensor("idxs", [PB, NG * (NIDX // 16)], I16,
                             kind="ExternalInput").ap()
    m01_in = nc.dram_tensor("m01", [PB, L], F32, kind="ExternalInput").ap()
    raw_out = nc.dram_tensor("raw", [PB, 1], F32, kind="ExternalOutput").ap()
    ginv_out = nc.dram_tensor("ginv", [128, NCH * PB], F32,
                              kind="ExternalOutput").ap()
    with tile.TileContext(nc) as tcx:
        with ExitStack() as ctx:
            _emit(ctx, tcx, y_in, idxs_in, m01_in, raw_out, ginv_out)
    nc.compile()
    return nc


def _run(in_maps, **kwargs):
    if "nc" not in _CACHE:
        _CACHE["nc"] = _build()
    return run_bass_kernel_spmd(_CACHE["nc"], in_maps,
                                core_ids=list(range(NCORES)), **kwargs)


def kernel(y_true: np.ndarray, y_pred: np.ndarray, **run_kwargs) -> np.ndarray:
    assert y_pred.shape == (B, T, C), y_pred.shape
    in_maps = []
    for c in range(NCORES):
        sl = slice(c * PB, (c + 1) * PB)
        prep = _host_prep(y_true[sl])
        in_maps.append({"ypred": np.ascontiguousarray(y_pred[sl], np.float32),
                        "idxs": prep["idxs"], "m01": prep["m01"]})
    res = _run(in_maps, **run_kwargs)
    raw = np.concatenate([res.results[c]["raw"] for c in range(NCORES)],
                         axis=0)[:, 0].astype(np.float64)
    # ginv[t, h*PB + g*G + i] for local sample b = g*G + i
    lng = np.zeros((B,), np.float64)
    for c in range(NCORES):
        gv = res.results[c]["ginv"].astype(np.float64)   # [128, 256]
        lng[c * PB:(c + 1) * PB] = np.log(
            gv.reshape(128, NCH, PB)).sum(axis=(0, 1))
    loss = -(np.log(raw) - lng)
    if run_kwargs:
        kernel.last_results = res  # expose trace info to test harness
    return loss[:, None].astype(np.float32)


# revision 20
# speedup vs baseline: 1.1904x; 1.0263x over previous
"""CTC loss (keras ctc_batch_cost semantics) on 8 Trainium2 NeuronCores.

Data-parallel over batch: 1024 samples -> 8 cores x 128 samples.
Host prep is integer-only (gather index tables + skip masks); all float
work runs on device.

Device pipeline (per core), built to minimize DMA instruction AND
descriptor count (HWDGE issue ~630ns per dma_start; the queue streams
descriptors at only ~8ns each, so descriptors must be >=2KB):

  A. 16 loads of y_pred in "core-major" layout: tile k holds samples
     [8k, 8k+8), partition p = (b_local, t_grp16), free = (t_sub16, c).
     Every descriptor is a contiguous 8KB DRAM run -> full HBM bandwidth
     with only 2048 descriptors total.
  B. ap_gather on GPSIMD: each of the 8 Q7 cores owns exactly one
     sample (its 16 partitions = 16 t-groups), so the per-core shared
     index list [65 blocks x 16 t_sub] is per-sample.  gt_k [p, (blk, ts)].
  C. per-column scale in gather layout: gmax over a 9-block subset per
     (sample, t), one broadcast scalar_tensor_tensor applies
     (p + EPS) * (e^RHAT / gmax).
  D. two PE-transpose hops reassemble sample-major ptil [b, blk, T]:
     (1) [p=(b8,tg), f=(blk8,ts)] -> PSUM [(blk8,ts), (b8,tg)], copied
     (bf16) into st_g [128, (k, b8, tg)]; (2) per (blk-group, tg):
     [(blk8,ts), b128] -> PSUM [b128, (blk8,ts)] -> ptil.
  E. s-sweep over the 129-row extended CTC lattice: each row's recursion
     v_t = (e_t + v_{t-1}) * p_t is ONE tensor_tensor_scan over the full
     T=256 (no mid renorm; the per-column scaling keeps fp32 range:
     max |v| ~ 6.7e20, min lsum ~ 1e-27 over the batch).  Row storage is
     a 5-slot ring buffer instead of the full [129, T] series.
Host assembles loss = -(log lsum - sum log ginv) in f64.
"""
from contextlib import ExitStack

import numpy as np

import concourse.bass as bass
import concourse.tile as tile
from concourse import bacc, mybir
from concourse.bass_utils import run_bass_kernel_spmd
from concourse.masks import make_identity

F32 = mybir.dt.float32
BF16 = mybir.dt.bfloat16
I16 = mybir.dt.int16
AF = mybir.ActivationFunctionType
ALU = mybir.AluOpType

B, T, C, L = 1024, 256, 128, 64
S = 2 * L + 1          # 129 extended states
NBLK = L + 1           # 64 label blocks + 1 blank block
BLANK = C - 1
EPS = 1e-7
RHAT = 0.2             # per-step prob boost: ginv = e^RHAT / gmax
PB = 128               # samples per core
BT = 8                 # samples per load tile (one per Q7 core)
NT = PB // BT          # 16 tiles
TG = 16                # t-groups per sample (partitions)
TS = T // TG           # 16 t-steps per group (in free dim)
NIDX = NBLK * TS       # gather indices per tile (1040)
NBP = NBLK + 1         # idx-table columns per tile, padded so every tile's
                       # slice starts 4-byte aligned (ap_gather reads the
                       # int16 index rows through a 32-bit word stream)
NG8 = 8                # full 8-block groups (blks 0..63); blank separate
RING = 5               # ring slots: 1 zero slot + 4 live state series
W = T + 1              # ring slot width: col0 = v_{-1} = 0, col 1+t = v_t
NCORES = 8


def _host_prep(y_true_shard: np.ndarray):
    yt = y_true_shard.astype(np.int64)
    idxs = np.zeros((PB, NT * NBP), np.int16)
    for k in range(NT):
        for q in range(BT):
            b = k * BT + q
            cls = np.full((NBLK,), BLANK, np.int64)
            cls[:L] = yt[b]
            flat = (np.arange(TS)[None, :] * C + cls[:, None]).reshape(-1)
            idxs[q * TG:(q + 1) * TG, k * NBP:k * NBP + NBLK] = \
                flat.reshape(NBLK, TG).T
    m01 = np.ones((PB, L), np.float32)
    m01[:, 1:] = (yt[:, 1:] != yt[:, :-1]).astype(np.float32)
    m01[:, 0] = 0.0
    return {"idxs": idxs, "m01": m01}


DEBUG_DUMPS = False


def _emit(ctx: ExitStack, tc: tile.TileContext, y_in, idxs_in, m01_in,
          raw_out, ginv_out, dbg=None):
    nc = tc.nc

    persist = ctx.enter_context(tc.tile_pool(name="persist", bufs=1))
    srcp = ctx.enter_context(tc.tile_pool(name="src", bufs=3))
    gtp = ctx.enter_context(tc.tile_pool(name="gt", bufs=3))
    small = ctx.enter_context(tc.tile_pool(name="small", bufs=4))
    cpool = ctx.enter_context(tc.tile_pool(name="cbuf", bufs=4))
    psA = ctx.enter_context(
        tc.tile_pool(name="psA", bufs=3, space=bass.MemorySpace.PSUM))
    psB = ctx.enter_context(
        tc.tile_pool(name="psB", bufs=3, space=bass.MemorySpace.PSUM))

    idxs = persist.tile([PB, NT * NBP], I16)
    nc.sync.dma_start(idxs[:], idxs_in[:])
    m01 = persist.tile([PB, L], F32)
    nc.sync.dma_start(m01[:], m01_in[:])

    ident32 = persist.tile([128, 128], F32)
    make_identity(nc, ident32[:])
    ident16 = persist.tile([128, 128], BF16)
    make_identity(nc, ident16[:])

    # staging: st_g[g8][p=(blk8, ts), f=(k, b8, tg)] bf16; blank on 16 parts
    st = [persist.tile([128, PB * TG], BF16, tag=f"st{g}", name=f"st{g}")
          for g in range(NG8)]
    st_bk = persist.tile([TG, PB * TG], BF16, tag="stbk")

    ptil = persist.tile([PB, NBLK * T], BF16)       # [b, blk, t]
    ring = persist.tile([PB, RING * W], F32)
    nc.gpsimd.memset(ring[:], 0.0)
    ginv_all = persist.tile([PB, NT * TS], F32)     # [(b8, tg), (k, ts)]
    raw = persist.tile([PB, 1], F32)

    # ---- feed: per 8-sample tile: load -> gather -> scale -> hop 1 ----
    for k in range(NT):
        src = srcp.tile([128, TS * C], F32, tag="src")
        nc.sync.dma_start(
            src[:].rearrange("p (ts c) -> p ts c", ts=TS),
            y_in[k * BT:(k + 1) * BT, :, :]
            .rearrange("b (tg ts) c -> (b tg) ts c", ts=TS))
        gt = gtp.tile([128, NIDX], F32, tag="gt")
        nc.gpsimd.ap_gather(
            gt[:], src[:], idxs[:, k * NBP:k * NBP + NBLK],
            channels=128, num_elems=TS * C, d=1, num_idxs=NIDX)
        # per-(sample, t) scale
        g3 = gt[:].rearrange("p (blk ts) -> p ts blk", blk=NBLK)
        gmax = small.tile([128, TS], F32, tag="gmax")
        nc.vector.tensor_reduce(gmax[:], g3[:, :, 0:NBLK:8],
                                axis=mybir.AxisListType.X, op=ALU.max)
        nc.vector.tensor_scalar_mul(gmax[:], gmax[:], float(np.exp(-RHAT)))
        gslice = ginv_all[:, k * TS:(k + 1) * TS]
        nc.vector.reciprocal(gslice, gmax[:])
        nc.vector.scalar_tensor_tensor(
            gt[:].rearrange("p (blk ts) -> p blk ts", blk=NBLK),
            gt[:].rearrange("p (blk ts) -> p blk ts", blk=NBLK),
            EPS,
            gslice[:, None, :].broadcast_to([128, NBLK, TS]),
            op0=ALU.add, op1=ALU.mult)
        # hop 1: [p=(b8,tg), (blk8, ts)] -> PSUM [(blk8,ts), (b8,tg)] -> st
        for g in range(NG8):
            pa = psA.tile([128, 128], F32, tag="pa")
            nc.tensor.transpose(pa[:], gt[:, g * 128:(g + 1) * 128], ident32[:])
            nc.scalar.activation(st[g][:, k * 128:(k + 1) * 128], pa[:],
                                 AF.Identity)
        pab = psA.tile([TG, 128], F32, tag="pab", bufs=1)
        nc.tensor.transpose(pab[:], gt[:, NG8 * 128:NG8 * 128 + TG], ident32[:])
        nc.scalar.activation(st_bk[:, k * 128:(k + 1) * 128], pab[:],
                             AF.Identity)
        if dbg is not None and k == 0:
            nc.sync.dma_start(dbg["gt"], gt[:])

    nc.sync.dma_start(ginv_out[:], ginv_all[:])

    # ---- hop 2: [(blk8, ts), b128 @ tg] -> PSUM [b, (blk8, ts)] -> ptil ----
    # blank first so the s-sweep can start as early as possible
    for tg in range(TG):
        pb = psB.tile([128, TG], BF16, tag="pbb", bufs=1)
        nc.tensor.transpose(pb[:], st_bk[:, tg::TG], ident16[0:TG, 0:TG])
        nc.scalar.activation(
            ptil[:, L * T + tg * TS: L * T + (tg + 1) * TS], pb[:], AF.Identity)
    for g in range(NG8):
        for tg in range(TG):
            pb = psB.tile([128, 128], BF16, tag="pb")
            nc.tensor.transpose(pb[:], st[g][:, tg::TG], ident16[:])
            nc.scalar.activation(
                ptil[:].rearrange("p (blk t) -> p blk t", blk=NBLK)
                [:, g * NG8:(g + 1) * NG8, tg * TS:(tg + 1) * TS],
                pb[:].rearrange("p (blk ts) -> p blk ts", blk=NG8),
                AF.Identity)

    # ---- phase D: 129 states, one full-T scan each, 5-slot ring ----
    def slot(s):  # ring slot columns for state s (s = -1 is the zero slot)
        r = (s + 1) % RING
        return ring[:, r * W:(r + 1) * W]

    for s in range(S):
        if s % 2 == 1:
            j = (s - 1) // 2
            c = cpool.tile([PB, T], F32, tag="c")
            nc.vector.scalar_tensor_tensor(
                c[:], slot(s - 2)[:, 0:T], m01[:, j:j + 1], slot(s - 1)[:, 0:T],
                op0=ALU.mult, op1=ALU.add)
            d0 = c[:]
            blk = j
        else:
            d0 = slot(s - 1)[:, 0:T]
            blk = L
        nc.vector.tensor_tensor_scan(
            slot(s)[:, 1:1 + T], d0, ptil[:, blk * T:(blk + 1) * T],
            1.0 if s <= 1 else 0.0,
            op0=ALU.add, op1=ALU.mult)

    nc.vector.tensor_add(raw[:, 0:1], slot(S - 2)[:, T:T + 1],
                         slot(S - 1)[:, T:T + 1])
    nc.sync.dma_start(raw_out[:], raw[:])
    if dbg is not None:
        nc.sync.dma_start(dbg["ptil"], ptil[:])


_CACHE: dict = {}


def _build():
    nc = bacc.Bacc("TRN2", target_bir_lowering=False, debug=False,
                   num_devices=NCORES, num_swdge_queues=4)
    y_in = nc.dram_tensor("ypred", [PB, T, C], F32, kind="ExternalInput").ap()
    idxs_in = nc.dram_tensor("idxs", [PB, NT * NBP], I16,
                             kind="ExternalInput").ap()
    m01_in = nc.dram_tensor("m01", [PB, L], F32, kind="ExternalInput").ap()
    raw_out = nc.dram_tensor("raw", [PB, 1], F32, kind="ExternalOutput").ap()
    ginv_out = nc.dram_tensor("ginv", [PB, NT * TS], F32,
                              kind="ExternalOutput").ap()
    dbg = None
    if DEBUG_DUMPS:
        dbg = {"gt": nc.dram_tensor("gt_dbg", [128, NIDX], F32,
                                    kind="ExternalOutput").ap(),
               "ptil": nc.dram_tensor("ptil_dbg", [PB, NBLK * T], BF16,
                                      kind="ExternalOutput").ap()}
    with tile.TileContext(nc) as tcx:
        with ExitStack() as ctx:
            _emit(ctx, tcx, y_in, idxs_in, m01_in, raw_out, ginv_out, dbg=dbg)
    nc.compile()
    return nc


def _run(in_maps, **kwargs):
    if "nc" not in _CACHE:
        _CACHE["nc"] = _build()
    return run_bass_kernel_spmd(_CACHE["nc"], in_maps,
                                core_ids=list(range(NCORES)), **kwargs)


def kernel(y_true: np.ndarray, y_pred: np.ndarray, **run_kwargs) -> np.ndarray:
    assert y_pred.shape == (B, T, C), y_pred.shape
    in_maps = []
    for c in range(NCORES):
        sl = slice(c * PB, (c + 1) * PB)
        prep = _host_prep(y_true[sl])
        in_maps.append({"ypred": np.ascontiguousarray(y_pred[sl], np.float32),
                        "idxs": prep["idxs"], "m01": prep["m01"]})
    res = _run(in_maps, **run_kwargs)
    raw = np.concatenate([res.results[c]["raw"] for c in range(NCORES)],
                         axis=0)[:, 0].astype(np.float64)
    # ginv[p=(q, tg), f=(k, ts)]; local sample b = k*BT + q, t = tg*TS + ts
    lng = np.zeros((B,), np.float64)
    for c in range(NCORES):
        gv = res.results[c]["ginv"].astype(np.float64)      # [128, 256]
        lg = np.log(gv).reshape(BT, TG, NT, TS)              # [q, tg, k, ts]
        lng[c * PB:(c + 1) * PB] = lg.sum(axis=(1, 3)).T.reshape(-1)
    loss = -(np.log(raw) - lng)
    if run_kwargs:
        kernel.last_results = res  # expose trace info to test harness
    return loss[:, None].astype(np.float32)


# revision 26
# speedup vs baseline: 2.0983x; 1.7626x over previous
"""CTC loss (keras ctc_batch_cost semantics) on 8 Trainium2 NeuronCores.

Data-parallel over batch: 1024 samples -> 8 cores x 128 samples.
Host prep is integer-only (gather index tables + skip masks); all float
work runs on device.

Device pipeline (per core), built to minimize DMA instruction AND
descriptor count (HWDGE issue ~630ns per dma_start; the queue streams
descriptors at only ~8ns each, so descriptors must be >=2KB):

  A. 16 loads of y_pred in "core-major" layout: tile k holds samples
     [8k, 8k+8), partition p = (b_local, t_grp16), free = (t_sub16, c).
     Every descriptor is a contiguous 8KB DRAM run -> full HBM bandwidth
     with only 2048 descriptors total.
  B. ap_gather on GPSIMD: each of the 8 Q7 cores owns exactly one
     sample (its 16 partitions = 16 t-groups), so the per-core shared
     index list [65 blocks x 16 t_sub] is per-sample.  gt_k [p, (blk, ts)].
  C. per-column scale in gather layout: gmax over a 9-block subset per
     (sample, t), one broadcast scalar_tensor_tensor applies
     (p + EPS) * (e^RHAT / gmax).
  D. two PE-transpose hops reassemble sample-major ptil [b, blk, T]:
     (1) [p=(b8,tg), f=(blk8,ts)] -> PSUM [(blk8,ts), (b8,tg)], copied
     (bf16) into st_g [128, (k, b8, tg)]; (2) per (blk-group, tg):
     [(blk8,ts), b128] -> PSUM [b128, (blk8,ts)] -> ptil.
  E. s-sweep over the 129-row extended CTC lattice: each row's recursion
     v_t = (e_t + v_{t-1}) * p_t is ONE tensor_tensor_scan over the full
     T=256 (no mid renorm; the per-column scaling keeps fp32 range:
     max |v| ~ 6.7e20, min lsum ~ 1e-27 over the batch).  Row storage is
     a 5-slot ring buffer instead of the full [129, T] series.
Host assembles loss = -(log lsum - sum log ginv) in f64.
"""
from contextlib import ExitStack

import numpy as np

import concourse.bass as bass
import concourse.tile as tile
from concourse import bacc, mybir
from concourse.bass_utils import run_bass_kernel_spmd
from concourse.masks import make_identity

F32 = mybir.dt.float32
BF16 = mybir.dt.bfloat16
I16 = mybir.dt.int16
AF = mybir.ActivationFunctionType
ALU = mybir.AluOpType

B, T, C, L = 1024, 256, 128, 64
S = 2 * L + 1          # 129 extended states
NBLK = L + 1           # 64 label blocks + 1 blank block
BLANK = C - 1
EPS = 1e-7
RHAT = 0.2             # per-step prob boost: ginv = e^RHAT / gmax
PB = 128               # samples per core
BT = 8                 # samples per load tile (one per Q7 core)
NT = PB // BT          # 16 tiles
TG = 16                # t-groups per sample (partitions)
TS = T // TG           # 16 t-steps per group (in free dim)
NBLKP = 80             # gather blocks padded: 65 real + 15 dummy, so that
                       # num_idxs = NBLKP is a multiple of 16
NIC = NBLKP // 16      # idx-table columns per tile actually read (5)
NBP = 8                # idx-table columns allocated per tile, so every
                       # tile's slice starts 4-byte aligned (ap_gather reads
                       # the int16 index rows through a 32-bit word stream)
NG8 = 8                # full 8-block groups (blks 0..63); blank separate
RING = 5               # ring slots: 1 zero slot + 4 live state series
W = T + 1              # ring slot width: col0 = v_{-1} = 0, col 1+t = v_t
NCORES = 8


def _host_prep(y_true_shard: np.ndarray):
    yt = y_true_shard.astype(np.int64)
    idxs = np.zeros((PB, NT * NBP), np.int16)
    for k in range(NT):
        for q in range(BT):
            b = k * BT + q
            cls = np.zeros((NBLKP,), np.int64)
            cls[:L] = yt[b]
            cls[L] = BLANK
            idxs[q * TG:(q + 1) * TG, k * NBP:k * NBP + NIC] = \
                cls.reshape(NIC, TG).T
    m01 = np.ones((PB, L), np.float32)
    m01[:, 1:] = (yt[:, 1:] != yt[:, :-1]).astype(np.float32)
    m01[:, 0] = 0.0
    return {"idxs": idxs, "m01": m01}


DEBUG_DUMPS = False


def _emit(ctx: ExitStack, tc: tile.TileContext, y_in, idxs_in, m01_in,
          raw_out, ginv_out, dbg=None):
    nc = tc.nc

    persist = ctx.enter_context(tc.tile_pool(name="persist", bufs=1))
    srcp = ctx.enter_context(tc.tile_pool(name="src", bufs=3))
    gtp = ctx.enter_context(tc.tile_pool(name="gt", bufs=3))
    small = ctx.enter_context(tc.tile_pool(name="small", bufs=4))
    cpool = ctx.enter_context(tc.tile_pool(name="cbuf", bufs=4))
    psA = ctx.enter_context(
        tc.tile_pool(name="psA", bufs=3, space=bass.MemorySpace.PSUM))
    psB = ctx.enter_context(
        tc.tile_pool(name="psB", bufs=3, space=bass.MemorySpace.PSUM))

    idxs = persist.tile([PB, NT * NBP], I16)
    nc.sync.dma_start(idxs[:], idxs_in[:])
    m01 = persist.tile([PB, L], F32)
    nc.sync.dma_start(m01[:], m01_in[:])

    ident16 = persist.tile([128, 128], BF16)
    make_identity(nc, ident16[:])

    # staging: st_g[g8][p=(blk8, ts), f=(k, b8, tg)] bf16; blank on 16 parts
    st = [persist.tile([128, PB * TG], BF16, tag=f"st{g}", name=f"st{g}")
          for g in range(NG8)]
    st_bk = persist.tile([TG, PB * TG], BF16, tag="stbk")

    ptil = persist.tile([PB, NBLK * T], BF16)       # [b, blk, t]
    ring = persist.tile([PB, RING * W], F32)
    nc.gpsimd.memset(ring[:], 0.0)
    ginv_all = persist.tile([PB, NT * TS], F32)     # [(b8, tg), (k, ts)]
    raw = persist.tile([PB, 1], F32)

    # ---- feed: per 8-sample tile: load -> gather -> scale -> hop 1 ----
    for k in range(NT):
        src = srcp.tile([128, TS * C], F32, tag="src")
        nc.sync.dma_start(
            src[:].rearrange("p (ts c) -> p ts c", ts=TS),
            y_in[k * BT:(k + 1) * BT, :, :]
            .rearrange("b (tg ts) c -> (b tg) ts c", ts=TS))
        # free-dim transpose [ts, c] -> [c, ts] so the gather can pull 16
        # contiguous floats per class (d=16: ~20x fewer Q7 read requests)
        srcT = srcp.tile([128, TS * C], F32, tag="srcT")
        nc.gpsimd.tensor_copy(
            srcT[:].rearrange("p (c ts) -> p c ts", ts=TS),
            src[:].rearrange("p (ts c) -> p c ts", ts=TS))
        gt = gtp.tile([128, NBLKP * TS], F32, tag="gt")
        nc.gpsimd.ap_gather(
            gt[:], srcT[:], idxs[:, k * NBP:k * NBP + NIC],
            channels=128, num_elems=C, d=TS, num_idxs=NBLKP)
        # per-(sample, t) scale
        g3 = gt[:].rearrange("p (blk ts) -> p ts blk", blk=NBLKP)
        gmax = small.tile([128, TS], F32, tag="gmax")
        nc.vector.tensor_reduce(gmax[:], g3[:, :, 0:NBLK:8],
                                axis=mybir.AxisListType.X, op=ALU.max)
        nc.vector.tensor_scalar_mul(gmax[:], gmax[:], float(np.exp(-RHAT)))
        gslice = ginv_all[:, k * TS:(k + 1) * TS]
        nc.vector.reciprocal(gslice, gmax[:])
        gts = gtp.tile([128, NBLK * TS], BF16, tag="gts")
        nc.vector.scalar_tensor_tensor(
            gts[:].rearrange("p (blk ts) -> p blk ts", blk=NBLK),
            gt[:, :NBLK * TS].rearrange("p (blk ts) -> p blk ts", blk=NBLK),
            EPS,
            gslice[:, None, :].broadcast_to([128, NBLK, TS]),
            op0=ALU.add, op1=ALU.mult)
        # hop 1: [p=(b8,tg), (blk8, ts)] -> PSUM [(blk8,ts), (b8,tg)] -> st
        for g in range(NG8):
            pa = psA.tile([128, 128], BF16, tag="pa")
            nc.tensor.transpose(pa[:], gts[:, g * 128:(g + 1) * 128],
                                ident16[:])
            nc.scalar.activation(st[g][:, k * 128:(k + 1) * 128], pa[:],
                                 AF.Identity)
        pab = psA.tile([TG, 128], BF16, tag="pab", bufs=1)
        nc.tensor.transpose(pab[:], gts[:, NG8 * 128:NG8 * 128 + TG],
                            ident16[:])
        nc.scalar.activation(st_bk[:, k * 128:(k + 1) * 128], pab[:],
                             AF.Identity)
        if dbg is not None and k == 0:
            nc.sync.dma_start(dbg["gt"], gt[:, :NBLK * TS])

    nc.sync.dma_start(ginv_out[:], ginv_all[:])

    # ---- hop 2: [(blk8, ts), b128 @ tg] -> PSUM [b, (blk8, ts)] -> ptil ----
    # blank first so the s-sweep can start as early as possible
    for tg in range(TG):
        pb = psB.tile([128, TG], BF16, tag="pbb", bufs=1)
        nc.tensor.transpose(pb[:], st_bk[:, tg::TG], ident16[0:TG, 0:TG])
        nc.scalar.activation(
            ptil[:, L * T + tg * TS: L * T + (tg + 1) * TS], pb[:], AF.Identity)
    for g in range(NG8):
        for tg in range(TG):
            pb = psB.tile([128, 128], BF16, tag="pb")
            nc.tensor.transpose(pb[:], st[g][:, tg::TG], ident16[:])
            nc.scalar.activation(
                ptil[:].rearrange("p (blk t) -> p blk t", blk=NBLK)
                [:, g * NG8:(g + 1) * NG8, tg * TS:(tg + 1) * TS],
                pb[:].rearrange("p (blk ts) -> p blk ts", blk=NG8),
                AF.Identity)

    # ---- phase D: 129 states, one full-T scan each, 5-slot ring ----
    def slot(s):  # ring slot columns for state s (s = -1 is the zero slot)
        r = (s + 1) % RING
        return ring[:, r * W:(r + 1) * W]

    for s in range(S):
        if s % 2 == 1:
            j = (s - 1) // 2
            c = cpool.tile([PB, T], F32, tag="c")
            nc.vector.scalar_tensor_tensor(
                c[:], slot(s - 2)[:, 0:T], m01[:, j:j + 1], slot(s - 1)[:, 0:T],
                op0=ALU.mult, op1=ALU.add)
            d0 = c[:]
            blk = j
        else:
            d0 = slot(s - 1)[:, 0:T]
            blk = L
        nc.vector.tensor_tensor_scan(
            slot(s)[:, 1:1 + T], d0, ptil[:, blk * T:(blk + 1) * T],
            1.0 if s <= 1 else 0.0,
            op0=ALU.add, op1=ALU.mult)

    nc.vector.tensor_add(raw[:, 0:1], slot(S - 2)[:, T:T + 1],
                         slot(S - 1)[:, T:T + 1])
    nc.sync.dma_start(raw_out[:], raw[:])
    if dbg is not None:
        nc.sync.dma_start(dbg["ptil"], ptil[:])


_CACHE: dict = {}


def _build():
    nc = bacc.Bacc("TRN2", target_bir_lowering=False, debug=False,
                   num_devices=NCORES, num_swdge_queues=4)
    y_in = nc.dram_tensor("ypred", [PB, T, C], F32, kind="ExternalInput").ap()
    idxs_in = nc.dram_tensor("idxs", [PB, NT * NBP], I16,
                             kind="ExternalInput").ap()
    m01_in = nc.dram_tensor("m01", [PB, L], F32, kind="ExternalInput").ap()
    raw_out = nc.dram_tensor("raw", [PB, 1], F32, kind="ExternalOutput").ap()
    ginv_out = nc.dram_tensor("ginv", [PB, NT * TS], F32,
                              kind="ExternalOutput").ap()
    dbg = None
    if DEBUG_DUMPS:
        dbg = {"gt": nc.dram_tensor("gt_dbg", [128, NBLK * TS], F32,
                                    kind="ExternalOutput").ap(),
               "ptil": nc.dram_tensor("ptil_dbg", [PB, NBLK * T], BF16,
                                      kind="ExternalOutput").ap()}
    with tile.TileContext(nc) as tcx:
        with ExitStack() as ctx:
            _emit(ctx, tcx, y_in, idxs_in, m01_in, raw_out, ginv_out, dbg=dbg)
    nc.compile()
    return nc


def _run(in_maps, **kwargs):
    if "nc" not in _CACHE:
        _CACHE["nc"] = _build()
    return run_bass_kernel_spmd(_CACHE["nc"], in_maps,
                                core_ids=list(range(NCORES)), **kwargs)


def kernel(y_true: np.ndarray, y_pred: np.ndarray, **run_kwargs) -> np.ndarray:
    assert y_pred.shape == (B, T, C), y_pred.shape
    in_maps = []
    for c in range(NCORES):
        sl = slice(c * PB, (c + 1) * PB)
        prep = _host_prep(y_true[sl])
        in_maps.append({"ypred": np.ascontiguousarray(y_pred[sl], np.float32),
                        "idxs": prep["idxs"], "m01": prep["m01"]})
    res = _run(in_maps, **run_kwargs)
    raw = np.concatenate([res.results[c]["raw"] for c in range(NCORES)],
                         axis=0)[:, 0].astype(np.float64)
    # ginv[p=(q, tg), f=(k, ts)]; local sample b = k*BT + q, t = tg*TS + ts
    lng = np.zeros((B,), np.float64)
    for c in range(NCORES):
        gv = res.results[c]["ginv"].astype(np.float64)      # [128, 256]
        lg = np.log(gv).reshape(BT, TG, NT, TS)              # [q, tg, k, ts]
        lng[c * PB:(c + 1) * PB] = lg.sum(axis=(1, 3)).T.reshape(-1)
    loss = -(np.log(raw) - lng)
    if run_kwargs:
        kernel.last_results = res  # expose trace info to test harness
    return loss[:, None].astype(np.float32)
